# revision 1
# baseline (speedup 1.0000x reference)
"""Bucket-indexed spatially-varying (channel-shared) 5x5 convolution on 8 trn2 cores.

out[b,c,y,x] = sum_{i,j} pad(input)[b,c,y+i,x+j] * kernel_bank[buckets[b,y,x], i, j]

Strategy (data-parallel over batch, one image per core):
  * Layout: partition dim = image row y (two 128-row tiles), free dim = (channel, x).
  * Per-pixel kernels ("Wmap") built on device: buckets -> one-hot (DVE is_equal)
    -> PE matmul against the [64,25] bank -> [25, Npix] tap-major weight map,
    staged to DRAM as [y, tap, x].
  * Conv: for each tap and each x column, one fused scalar_tensor_tensor op:
      acc[y, :, x] = (x_shift[y, :, x+dx] * w[y]) + acc[y, :, x]
    where w is a per-partition (per-row) scalar AP - exact fp32 FMA in one
    DVE pass. dy shifts are handled by loading 5 row-shifted copies of the
    input tile (partition shifts are impossible inside SBUF ops).
"""

import sys

sys.path.insert(0, "/opt/trn_rl_repo")

import numpy as np

B, C, H, W = 8, 128, 256, 256
K, NB = 5, 64
PAD = (K - 1) // 2  # 2
HP, WP = H + 2 * PAD, W + 2 * PAD  # 260, 260
N_CORES = 8
NT = K * K  # 25 taps

YT = 2  # y tiles of 128 rows
XW = 16  # x block width
NXB = W // XW  # 16

_CACHE = {}


def _build_nc(conv_reps=1):
    import concourse.bacc as bacc
    import concourse.mybir as mybir
    from concourse import tile

    f32 = mybir.dt.float32
    Alu = mybir.AluOpType

    nc = bacc.Bacc(None)

    # channel-minor layouts (host transposes): contiguous DMA bursts
    xp = nc.dram_tensor("xp", [HP, WP, C], f32, kind="ExternalInput")
    bkf = nc.dram_tensor("bkf", [H, W], f32, kind="ExternalInput")
    bank = nc.dram_tensor("bank", [NB, NT], f32, kind="ExternalInput")
    iota = nc.dram_tensor("iota", [NB, 1], f32, kind="ExternalInput")
    y_out = nc.dram_tensor("y", [H, W, C], f32, kind="ExternalOutput")

    with tile.TileContext(nc) as tc:
        with tc.tile_pool(name="dram", bufs=1, space="DRAM") as dpool:
            # weight map staged in DRAM as [y, tap, x]
            wm_dram = dpool.tile([H, NT, W], f32)

            # ---------------- Phase A: build Wmap ----------------
            with (
                tc.tile_pool(name="wconst", bufs=1) as cpool,
                tc.tile_pool(name="wbuild", bufs=3) as wpool,
                tc.tile_pool(name="wpsum", bufs=4, space="PSUM") as pspool,
            ):
                bank_sb = cpool.tile([NB, NT], f32)
                nc.sync.dma_start(out=bank_sb[:], in_=bank[:])
                iota_sb = cpool.tile([NB, 1], f32)
                nc.sync.dma_start(out=iota_sb[:], in_=iota[:])

                GROWS = 16  # bucket rows per group
                GPIX = GROWS * W  # 4096
                for g in range(H // GROWS):  # 16 groups
                    brep = wpool.tile([NB, GPIX], f32, tag="brep")
                    nc.sync.dma_start(
                        out=brep[:],
                        in_=bkf[g * GROWS : (g + 1) * GROWS, :]
                        .rearrange("(o h) w -> o (h w)", o=1)
                        .broadcast_to((NB, GPIX)),
                    )
                    oh = wpool.tile([NB, GPIX], f32, tag="oh")
                    nc.vector.tensor_scalar(
                        out=oh[:],
                        in0=brep[:],
                        scalar1=iota_sb[:],
                        scalar2=None,
                        op0=Alu.is_equal,
                    )
                    for c8 in range(GPIX // 512):  # 8 chunks of 512 px (2 rows)
                        ps = pspool.tile([NT, 512], f32, tag="ps")
                        nc.tensor.matmul(
                            ps[:],
                            bank_sb[:],
                            oh[:, c8 * 512 : (c8 + 1) * 512],
                            start=True,
                            stop=True,
                        )
                        wms = wpool.tile([NT, 512], f32, tag="wms")
                        nc.scalar.copy(out=wms[:], in_=ps[:])
                        y0 = g * GROWS + c8 * 2
                        # keep the SBUF partition dim (t) first on both sides;
                        # an SBUF-side rearrange that moves the partition dim
                        # scrambles the transfer.
                        nc.sync.dma_start(
                            out=wm_dram[y0 : y0 + 2, :, :].rearrange(
                                "y t x -> t y x"
                            ),
                            in_=wms.rearrange("t (y x) -> t y x", y=2),
                        )

            # ---------------- Phase B: convolution ----------------
            with (
                tc.tile_pool(name="xs", bufs=2) as xpool,
                tc.tile_pool(name="wm", bufs=2) as wmpool,
                tc.tile_pool(name="acc", bufs=2) as apool,
            ):
                for rep in range(conv_reps):
                  for yt in range(YT):
                    for xb in range(NXB):
                        x0 = xb * XW
                        wm_t = wmpool.tile([128, NT, XW], f32, tag="wm")
                        nc.sync.dma_start(
                            out=wm_t[:],
                            in_=wm_dram[yt * 128 : (yt + 1) * 128, :, x0 : x0 + XW],
                        )
                        xs = xpool.tile([128, K, XW + 2 * PAD, C], f32, tag="xs")
                        for dy in range(K):
                            nc.sync.dma_start(
                                out=xs[:, dy, :, :],
                                in_=xp[
                                    yt * 128 + dy : yt * 128 + dy + 128,
                                    x0 : x0 + XW + 2 * PAD,
                                    :,
                                ],
                            )
                        acc = apool.tile([128, XW, C], f32, tag="acc")
                        for t in range(NT):
                            dy, dx = t // K, t % K
                            for ix in range(XW):
                                in0 = xs[:, dy, ix + dx, :]
                                w = wm_t[:, t, ix : ix + 1]
                                if t == 0:
                                    nc.vector.tensor_scalar(
                                        out=acc[:, ix, :],
                                        in0=in0,
                                        scalar1=w,
                                        scalar2=None,
                                        op0=Alu.mult,
                                    )
                                else:
                                    nc.vector.scalar_tensor_tensor(
                                        out=acc[:, ix, :],
                                        in0=in0,
                                        scalar=w,
                                        in1=acc[:, ix, :],
                                        op0=Alu.mult,
                                        op1=Alu.add,
                                    )
                        nc.sync.dma_start(
                            out=y_out[
                                yt * 128 : (yt + 1) * 128, x0 : x0 + XW, :
                            ],
                            in_=acc[:],
                        )

    nc.finalize()
    return nc


def _get_nc():
    if "nc" not in _CACHE:
        _CACHE["nc"] = _build_nc()
    return _CACHE["nc"]


def kernel(input, kernel_bank, buckets):
    from concourse.bass_utils import run_bass_kernel_spmd

    nc = _get_nc()

    input = np.ascontiguousarray(input, dtype=np.float32)
    # pad spatially, then channel-minor [b, y, x, c] for contiguous DMA
    xpad = np.pad(input, ((0, 0), (0, 0), (PAD, PAD), (PAD, PAD)))
    xpad = np.ascontiguousarray(xpad.transpose(0, 2, 3, 1))
    bkf = np.ascontiguousarray(buckets, dtype=np.int32).astype(np.float32)
    bank2 = np.ascontiguousarray(kernel_bank, dtype=np.float32).reshape(NB, NT)
    iota64 = np.arange(NB, dtype=np.float32).reshape(NB, 1)

    in_maps = [
        {"xp": xpad[i], "bkf": bkf[i], "bank": bank2, "iota": iota64}
        for i in range(N_CORES)
    ]
    res = run_bass_kernel_spmd(nc, in_maps, list(range(N_CORES)))
    # device output is [H, W, C]; back to [C, H, W]
    out = np.stack(
        [res.results[i]["y"].transpose(2, 0, 1) for i in range(N_CORES)], axis=0
    )
    return np.ascontiguousarray(out, dtype=np.float32)



# revision 4
# speedup vs baseline: 3.2849x; 3.2849x over previous
"""Bucket-indexed spatially-varying (channel-shared) 5x5 convolution on 8 trn2 cores.

out[b,c,y,x] = sum_{i,j} pad(input)[b,c,y+i,x+j] * kernel_bank[buckets[b,y,x], i, j]

Strategy (data-parallel over batch, one image per core), all bf16 on device:
  * Phase A: buckets -> one-hot (DVE is_equal) -> PE matmul against the
    [64,25] bank -> per-pixel weight map wm staged to DRAM as [y, tap, x].
  * Phase B layout: partition = output row y (2 chunks of 128), free =
    (c, x) with x minor. Five row-shifted copies of the input tile make
    the dy shift a partition-aligned read; dx is a free-dim offset.
  * Per tap: ONE big DVE tensor_tensor mult (weight map broadcast across
    c via a stride-0 AP; x-minor keeps every operand packed bf16 so the
    DVE 2x perf mode engages):
        P[y, c, x] = xs[y+i, c, x+j] * wm[y, t, x]
    The 25 tap products are accumulated on the otherwise-idle PE with
    identity-stationary matmuls into PSUM (fp32), then evicted to bf16
    by the ACT engine. Host converts bf16 -> fp32.
"""

import sys

sys.path.insert(0, "/opt/trn_rl_repo")

import numpy as np

B, C, H, W = 8, 128, 256, 256
K, NB = 5, 64
PAD = (K - 1) // 2  # 2
HP, WP = H + 2 * PAD, W + 2 * PAD  # 260, 260
N_CORES = 8
NT = K * K  # 25 taps

CBLK = 16  # channel block
NCB = C // CBLK  # 8
XH = 128  # x half width
NXH = W // XH  # 2

_CACHE = {}


def _build_nc():
    import concourse.bacc as bacc
    import concourse.mybir as mybir
    from concourse import tile

    f32 = mybir.dt.float32
    bf16 = mybir.dt.bfloat16
    Alu = mybir.AluOpType
    Act = mybir.ActivationFunctionType

    nc = bacc.Bacc(None)

    # channel-mid layout [row, c, x]: per-partition contiguous c-block rows
    xp = nc.dram_tensor("xp", [HP, C, WP], bf16, kind="ExternalInput")
    bkf = nc.dram_tensor("bkf", [H, W], bf16, kind="ExternalInput")
    bank = nc.dram_tensor("bank", [NB, NT], bf16, kind="ExternalInput")
    iota = nc.dram_tensor("iota", [NB, 1], f32, kind="ExternalInput")
    ident = nc.dram_tensor("ident", [128, 128], bf16, kind="ExternalInput")
    y_out = nc.dram_tensor("y", [H, C, W], bf16, kind="ExternalOutput")

    with tile.TileContext(nc) as tc:
        with tc.tile_pool(name="dram", bufs=1, space="DRAM") as dpool:
            # weight map staged in DRAM as [y, tap, x]
            wm_dram = dpool.tile([H, NT, W], bf16)

            # ---------------- Phase A: build Wmap ----------------
            with (
                tc.tile_pool(name="wconst", bufs=1) as cpool,
                tc.tile_pool(name="wbuild", bufs=3) as wpool,
                tc.tile_pool(name="wpsum", bufs=1, space="PSUM") as pspool,
            ):
                bank_sb = cpool.tile([NB, NT], bf16)
                nc.sync.dma_start(out=bank_sb[:], in_=bank[:])
                iota_sb = cpool.tile([NB, 1], f32)
                nc.sync.dma_start(out=iota_sb[:], in_=iota[:])

                GROWS = 16  # bucket rows per group
                GPIX = GROWS * W  # 4096
                for g in range(H // GROWS):  # 16 groups
                    brep = wpool.tile([NB, GPIX], bf16, tag="brep")
                    nc.sync.dma_start(
                        out=brep[:],
                        in_=bkf[g * GROWS : (g + 1) * GROWS, :]
                        .rearrange("(o h) w -> o (h w)", o=1)
                        .broadcast_to((NB, GPIX)),
                    )
                    oh = wpool.tile([NB, GPIX], bf16, tag="oh")
                    nc.vector.tensor_scalar(
                        out=oh[:],
                        in0=brep[:],
                        scalar1=iota_sb[:],
                        scalar2=None,
                        op0=Alu.is_equal,
                    )
                    ps = pspool.tile([NT, GPIX], f32, tag="ps")
                    for s in range(GPIX // 512):
                        nc.tensor.matmul(
                            ps[:, s * 512 : (s + 1) * 512],
                            bank_sb[:],
                            oh[:, s * 512 : (s + 1) * 512],
                            start=True,
                            stop=True,
                        )
                    wms = wpool.tile([NT, GPIX], bf16, tag="wms")
                    nc.scalar.copy(out=wms[:], in_=ps[:])
                    y0 = g * GROWS
                    # keep the SBUF partition dim (t) first on both sides;
                    # an SBUF-side rearrange that moves the partition dim
                    # scrambles the transfer.
                    nc.sync.dma_start(
                        out=wm_dram[y0 : y0 + GROWS, :, :].rearrange(
                            "y t x -> t y x"
                        ),
                        in_=wms.rearrange("t (y x) -> t y x", y=GROWS),
                    )

            # ---------------- Phase B: convolution ----------------
            with (
                tc.tile_pool(name="const", bufs=1) as kpool,
                tc.tile_pool(name="wm", bufs=2) as wmpool,
                tc.tile_pool(name="xs", bufs=2) as xpool,
                tc.tile_pool(name="prod", bufs=3) as ppool,
                tc.tile_pool(name="out", bufs=2) as opool,
                tc.tile_pool(name="psum", bufs=2, space="PSUM") as pspool,
            ):
                ident_sb = kpool.tile([128, 128], bf16)
                nc.sync.dma_start(out=ident_sb[:], in_=ident[:])

                for a in (0, 128):  # y chunk
                    wt = wmpool.tile([128, NT, W], bf16, tag="wt")
                    nc.sync.dma_start(
                        out=wt[:], in_=wm_dram[a : a + 128, :, :]
                    )
                    for cb in range(NCB):
                        c0 = cb * CBLK
                        xts = []
                        for i in range(K):
                            xt = xpool.tile(
                                [128, CBLK, WP], bf16, tag=f"xt{i}"
                            )
                            nc.sync.dma_start(
                                out=xt[:],
                                in_=xp[
                                    a + i : a + i + 128,
                                    c0 : c0 + CBLK,
                                    :,
                                ],
                            )
                            xts.append(xt)
                        for xh in range(NXH):
                            x0 = xh * XH
                            acc = pspool.tile([128, CBLK * XH], f32, tag="acc")
                            for t in range(NT):
                                i, j = t // K, t % K
                                p = ppool.tile([128, CBLK, XH], bf16, tag="p")
                                nc.vector.tensor_tensor(
                                    out=p[:],
                                    in0=xts[i][:, :, x0 + j : x0 + j + XH],
                                    in1=wt[:, t, x0 : x0 + XH]
                                    .unsqueeze(1)
                                    .broadcast_to((128, CBLK, XH)),
                                    op=Alu.mult,
                                )
                                pf = p.rearrange("p c x -> p (c x)")
                                for s in range(CBLK * XH // 512):
                                    nc.tensor.matmul(
                                        acc[:, s * 512 : (s + 1) * 512],
                                        ident_sb[:],
                                        pf[:, s * 512 : (s + 1) * 512],
                                        start=(t == 0),
                                        stop=(t == NT - 1),
                                    )
                            ot = opool.tile([128, CBLK * XH], bf16, tag="ot")
                            nc.scalar.copy(out=ot[:], in_=acc[:])
                            nc.sync.dma_start(
                                out=y_out[
                                    a : a + 128, c0 : c0 + CBLK, x0 : x0 + XH
                                ],
                                in_=ot.rearrange("p (c x) -> p c x", c=CBLK),
                            )

    nc.finalize()
    return nc


def _get_nc():
    if "nc" not in _CACHE:
        _CACHE["nc"] = _build_nc()
    return _CACHE["nc"]


def _make_in_maps(inputs):
    import concourse.mybir as mybir

    bf16 = mybir.dt.np(mybir.dt.bfloat16)

    x = np.ascontiguousarray(inputs["input"], dtype=np.float32)
    # pad spatially, then [b, row, c, x] channel-mid layout
    xpad = np.pad(x, ((0, 0), (0, 0), (PAD, PAD), (PAD, PAD)))
    xpad = np.ascontiguousarray(xpad.transpose(0, 2, 1, 3)).astype(bf16)
    bkf = np.ascontiguousarray(inputs["buckets"], dtype=np.int32).astype(
        np.float32
    ).astype(bf16)  # ids < 64: exact in bf16
    bank2 = (
        np.ascontiguousarray(inputs["kernel_bank"], dtype=np.float32)
        .reshape(NB, NT)
        .astype(bf16)
    )
    iota64 = np.arange(NB, dtype=np.float32).reshape(NB, 1)
    ident = np.eye(128, dtype=np.float32).astype(bf16)
    return [
        {
            "xp": xpad[i],
            "bkf": bkf[i],
            "bank": bank2,
            "iota": iota64,
            "ident": ident,
        }
        for i in range(N_CORES)
    ]


def kernel(input, kernel_bank, buckets):
    from concourse.bass_utils import run_bass_kernel_spmd

    nc = _get_nc()
    in_maps = _make_in_maps(
        {"input": input, "kernel_bank": kernel_bank, "buckets": buckets}
    )
    res = run_bass_kernel_spmd(nc, in_maps, list(range(N_CORES)))
    # device output is [H, C, W] bf16; back to [C, H, W] fp32
    out = np.stack(
        [
            res.results[i]["y"].astype(np.float32).transpose(1, 0, 2)
            for i in range(N_CORES)
        ],
        axis=0,
    )
    return np.ascontiguousarray(out, dtype=np.float32)
